# revision 35
# baseline (speedup 1.0000x reference)
"""2D DCT [8,32,256,256] on 8 TRN2 NeuronCores — raw Bass (no Tile).

Math: with A[m,k] = cos(pi*k*(m+0.5)/L)/L the 2D DCT per [256,256] slice is
    out = A^T @ X @ A
Stage 1: V = X^T A via 4 matmuls N=256 per slice (lhsT = X h-chunks,
rhs = A), one PSUM bank per slice. The host stages the second half of the
w columns REVERSED, so the bank holds
    vp[v, 0:256]   = v0 = V[v, j]        (v = 0..127)
    vp[v, 256:512] = v1 = V[255-v, j]
Stage 2 uses the DCT-II even/odd symmetry A[255-v, w'] = (-1)^w' A[v, w']:
    out[j, 2t']   = E2^T (v0 + v1),   E2[v,t'] = A[v, 2t']
    out[j, 2t'+1] = O2^T v0 - O2^T v1, O2[v,t'] = A[v, 2t'+1]
Per slice PAIR stage 2 is 3 matmuls of N=512 (contraction 128): the even
half consumes a DVE-folded s_w = v0+v1 (bf16, 2x-mode tensor_tensor); the
odd half does the subtract INSIDE PSUM accumulation using a staged -O2
(f32-exact, no fold needed). 1536+216 streamed PE columns per slice vs
2048 for the dense baseline, while the vector engines carry only
casts + one fold + out-evictions (~20us each, well under the PE's ~25us)
so the PE is self-paced — cross-engine hiccups don't propagate.

Pipeline per pair p (slices a=2p, b=2p+1):
    PE  S1(a), S1(b)          -> vp banks a%4, b%4  (4 MMs N=256 each)
    ACT cast(s) FD=512        vp bank -> vf[v0-group | v1-group] bf16
    DVE fold_s(p) FD=512 2x   vf v0,v1 -> vs_s (s_w pair, contiguous)
    PE  S2(p): E2^T s_w (N=512); O2^T v0pair - O2n^T v1pair (2 MMs N=512)
    DVE out-evict (ACT for pairs 3,9,15) op banks -> os bf16 FD=1024
    sync-ring DMA os -> DRAM (ACT DMAs the tail pair inline)

Wait discipline (waits break the LDWEIGHTS pull-ahead): PE block p =
[wait act>=cast(2p-3): vp two-agent guard, also implies S2(p-2)'s casts]
S1(2p) S1(2p+1) [wait dve>=out(p-4), which implies fold_s(p-2) by DVE
stream order] S2(p-2), LAG=2. ACT stream: cast(s) ascending. DVE:
fold_s(p) then out(p-2). The TAIL pairs (14, 15) write their stage-2
output into the by-then-free vp banks instead of the op ring, removing
the final out-evict -> S2 serialization from the kernel drain. Never two
agents on one PSUM bank concurrently (hard device crash) — every PSUM
reader/writer handoff above is sem-ordered.

Measured paces (this container, warm K=8/8 @2.4GHz): N=256 MM 109ns,
N=512 MM 216ns, ACT copy FD/1.2+143ns, DVE cast FD/0.96+65ns, DVE bf16
TT 2x FD/1.92+69ns. HAM: PE cold (1.2GHz) until ~3.4us of sustained
work — N_WARM garbage matmuls bridge the DMA head so real S1s start
warm. Dual-PSUM-operand tensor_tensor is ILLEGAL (walrus NCC_IBVF027);
DMA cannot read PSUM; matmul output must be f32 on TRN2 — these three
constraints shape the whole eviction pipeline.

Measured HW exec (neuron-profile, core 0): 43.4-44.7us across runs
(prior dense-baseline: 47.1-52.2us in the same container), rel err
3.7e-3. Breakdown at 43.4: ~1.4us counted boot, 3.5us HAM/DMA-head
warm, 25.9us compute window (24.3 PE-streaming floor + 1.4 stalls),
4.7us out-drain, 8.0us fixed runtime postamble (sem resets). Note the
device sometimes runs the PE at 2.0GHz (P0 power state) instead of
2.4GHz, which stretches the compute window ~20%.
"""

import numpy as np

import concourse.bacc as bacc
import concourse.bass as bass
import concourse.mybir as mybir
from concourse.bass_utils import run_bass_kernel_spmd

N_CORES = 8
C = 32                    # slices per core
P = 16                    # slice pairs per core
L = 256
BF16 = mybir.dt.bfloat16
F32 = mybir.dt.float32
NP_BF16 = mybir.dt.np(mybir.dt.bfloat16)

# staged input units: 0 = A, 1 = [E2|O2|O2n|pad], 2+s = slice s
IN_CHUNKS = [3, 1, 1, 1, 1, 1, 1, 1, 1, 2, 2, 3, 4, 5, 7]  # 34 units
OUT_CHUNKS = [3, 3, 3, 3, 2, 1]                   # pairs 0..14 on sync ring
TAIL_PAIR = 15                                    # pair 15 DMA'd from ACT
N_WARM = 36
VPR = 4                   # vp ring (banks) — slice s -> bank s%4
OPR = 4                   # op ring — pair p -> banks 2*(p%2), 2*(p%2)+1
VFR = 8                   # vf ring slots — slice s -> slot s%8
VSR = 6                   # vs_s ring — pair p -> slot p%6
LAG = 2                   # S2(p-LAG) in PE pair block p
OUT_ENG = ["act" if p == TAIL_PAIR else "dve" for p in range(P)]
# Tail odd casts on DVE were tried to parallelize the drain; DVE's
# strict-FIFO queue turned the hoisted cast's wait into head-of-line
# blocking and made the tail worse — keep all casts on ACT.
DVE_CASTS: set[int] = set()


def _dct_matrix() -> np.ndarray:
    m = np.arange(L, dtype=np.float64)
    k = np.arange(L, dtype=np.float64)
    a = np.cos(np.pi * np.outer(m + 0.5, k) / L) / L
    return a.astype(np.float32)


def _chunk_of_slice(s):
    u = s + 2
    c0 = 0
    for ci, n in enumerate(IN_CHUNKS):
        if u < c0 + n:
            return ci
        c0 += n
    raise AssertionError


def _schedules():
    """Per-engine op orders + completion counts (sem value when done)."""
    pe = []
    for p in range(P):
        pe.append(("S1", 2 * p))
        pe.append(("S1", 2 * p + 1))
        if p >= LAG:
            pe.append(("S2", p - LAG))
    for p in range(P - LAG, P):
        pe.append(("S2", p))
    pe_count = {o: i + 1 for i, o in enumerate(pe)}

    # ACT: casts ascending; out(q) placed right after cast(2q+3) so the
    # PE block's act>=cast(2p-3) wait transitively covers ACT outs.
    # The final pairs' odd casts (29, 31) run on DVE instead so the
    # kernel drain chain S1(31)->cast->fold->S2(15) isn't serialized
    # behind cast(30) on ACT.
    act = []
    for s in range(2 * P):
        if s in DVE_CASTS:
            continue
        act.append(("cast", s))
        if s >= 3 and s % 2 == 1:
            q = (s - 3) // 2
            if OUT_ENG[q] == "act":
                act.append(("out", q))
    for q in (P - 2, P - 1):
        if OUT_ENG[q] == "act":
            act.append(("out", q))
    act_count = {o: i + 1 for i, o in enumerate(act)}

    # DVE: fold_s(p) leads, out(p-2) trails
    dve = []
    for p in range(P):
        if 2 * p + 1 in DVE_CASTS:
            dve.append(("cast", 2 * p + 1))
        dve.append(("fold", p))
        q = p - 2
        if q >= 0 and OUT_ENG[q] == "dve":
            dve.append(("out", q))
    for q in (P - 2, P - 1):
        if OUT_ENG[q] == "dve":
            dve.append(("out", q))
    dve_count = {o: i + 1 for i, o in enumerate(dve)}
    return pe, pe_count, act, act_count, dve, dve_count


def _build(sim: bool = False) -> bass.Bass:
    nc = bacc.Bacc()
    x = nc.declare_dram_parameter("x", [128, 2 + C, 512], BF16, isOutput=False)
    out = nc.declare_dram_parameter("out", [128, P, 2, 512], BF16, isOutput=True)

    pe, pe_count, act, act_count, dve, dve_count = _schedules()

    from contextlib import ExitStack

    ctx = ExitStack()
    with ctx:
        warm_sb = ctx.enter_context(nc.sbuf_tensor([128, 128], BF16))
        xs = ctx.enter_context(nc.sbuf_tensor([128, 2 + C, 512], BF16))
        # vf[:, 0, slot, :] = v0 of slice, vf[:, 1, slot, :] = v1
        vf = ctx.enter_context(nc.sbuf_tensor([128, 2, VFR, 256], BF16))
        vs_s = ctx.enter_context(nc.sbuf_tensor([128, VSR, 2, 256], BF16))
        os_ = ctx.enter_context(nc.sbuf_tensor([128, P, 2, 512], BF16))
        vp = ctx.enter_context(nc.psum_tensor([128, VPR, 512], F32))
        op = ctx.enter_context(nc.psum_tensor([128, OPR, 512], F32))

        in_sems = [
            ctx.enter_context(nc.semaphore(f"in_sem{i}"))
            for i in range(len(IN_CHUNKS))
        ]
        pe_sem = ctx.enter_context(nc.semaphore("pe_sem"))
        dve_sem = ctx.enter_context(nc.semaphore("dve_sem"))
        act_sem = ctx.enter_context(nc.semaphore("act_sem"))
        out_sem = ctx.enter_context(nc.semaphore("out_sem"))
        warm_sem = ctx.enter_context(nc.semaphore("warm_sem"))
        sem_of = {"dve": dve_sem, "act": act_sem}
        count_of = {"dve": dve_count, "act": act_count}

        block = ctx.enter_context(nc.Block())

        @block.sync
        def _(eng):
            u0 = 0
            for ci, n in enumerate(IN_CHUNKS):
                eng.dma_start(
                    xs[:, u0 : u0 + n, :], x[:, u0 : u0 + n, :]
                ).then_inc(in_sems[ci], 16)
                u0 += n
            c0 = 0
            for n in OUT_CHUNKS:
                for eng_name in ("dve", "act"):
                    need = max(
                        (
                            count_of[eng_name][("out", q)]
                            for q in range(c0, c0 + n)
                            if OUT_ENG[q] == eng_name
                        ),
                        default=0,
                    )
                    if need:
                        eng.wait_ge(sem_of[eng_name], need)
                eng.dma_start(
                    out[:, c0 : c0 + n, :, :], os_[:, c0 : c0 + n, :, :]
                ).then_inc(out_sem, 16)
                c0 += n
            eng.wait_ge(out_sem, 16 * (len(OUT_CHUNKS) + 1))

        @block.tensor
        def _(eng):
            if sim:
                eng.wait_ge(warm_sem, 1)
            for _ in range(N_WARM):
                nc.tensor.matmul(
                    vp[:, 0, 0:128], warm_sb[:], warm_sb[:],
                    start=True, stop=True,
                )
            eng.wait_ge(in_sems[0], 16)
            seen_chunks = {0}
            for kind, i in pe:
                if kind == "S1":
                    s = i
                    ci = _chunk_of_slice(s)
                    if ci not in seen_chunks:
                        seen_chunks.add(ci)
                        eng.wait_ge(in_sems[ci], 16)
                    if s % 2 == 0 and s >= 4:
                        # vp two-agent guard: bank freed by cast(s-3);
                        # also implies everything S2(s//2 - 2) needs
                        # from the ACT stream
                        eng.wait_ge(act_sem, act_count[("cast", s - 3)])
                    r = s % VPR
                    for mi in range(2):
                        for ki in range(2):
                            mm = nc.tensor.matmul(
                                vp[:, r, mi * 256 : (mi + 1) * 256],
                                xs[:, 2 + s, ki * 256 + mi * 128 : ki * 256 + (mi + 1) * 128],
                                xs[:, 0, ki * 256 : (ki + 1) * 256],
                                start=(ki == 0),
                                stop=(ki == 1),
                            )
                    mm.then_inc(pe_sem, 1)
                else:
                    q = i
                    if q >= P - 2:
                        # tail pairs write into the (now free) vp banks:
                        # no op-ring wait; fold(q) implies cast(2q+1)
                        # which implies the banks' casts are done
                        eng.wait_ge(dve_sem, dve_count[("fold", q)])
                        ps, b0 = vp, 2 * (q - (P - 2))
                    elif q >= 2 and OUT_ENG[q - 2] == "dve":
                        # implies fold_s(q) done too (stream order)
                        eng.wait_ge(dve_sem, dve_count[("out", q - 2)])
                        ps, b0 = op, 2 * (q % 2)
                    else:
                        eng.wait_ge(dve_sem, dve_count[("fold", q)])
                        ps, b0 = op, 2 * (q % 2)
                    f0 = (2 * q) % VFR
                    nc.tensor.matmul(
                        ps[:, b0, :],
                        xs[:, 1, 0:128],
                        vs_s[:, q % VSR, :, :],
                        start=True, stop=True,
                    )
                    nc.tensor.matmul(
                        ps[:, b0 + 1, :],
                        xs[:, 1, 128:256],
                        vf[:, 0, f0 : f0 + 2, :],
                        start=True, stop=False,
                    )
                    mm = nc.tensor.matmul(
                        ps[:, b0 + 1, :],
                        xs[:, 1, 256:384],
                        vf[:, 1, f0 : f0 + 2, :],
                        start=False, stop=True,
                    )
                    mm.then_inc(pe_sem, 1)

        @block.scalar
        def _(eng):
            for kind, i in act:
                if kind == "cast":
                    s = i
                    eng.wait_ge(pe_sem, pe_count[("S1", s)])
                    cp = nc.scalar.copy(
                        vf[:, :, s % VFR, :],
                        vp[:, s % VPR, :],
                    )
                else:
                    q = i
                    eng.wait_ge(pe_sem, pe_count[("S2", q)])
                    if q >= P - 2:
                        src = vp[:, 2 * (q - (P - 2)) : 2 * (q - (P - 2)) + 2, :]
                    else:
                        src = op[:, 2 * (q % 2) : 2 * (q % 2) + 2, :]
                    cp = nc.scalar.copy(os_[:, q, :, :], src)
                cp.then_inc(act_sem, 1)
            # tail out-DMA for pair 15, in parallel with the sync ring's
            # pair-14 chunk; the DGE must not read os_ before the
            # eviction writes land, so wait on ACT's own eviction sem
            eng.wait_ge(act_sem, act_count[("out", P - 1)])
            eng.dma_start(
                out[:, P - 1, :, :], os_[:, P - 1, :, :]
            ).then_inc(out_sem, 16)

        @block.vector
        def _(eng):
            add = mybir.AluOpType.add
            if sim:
                nc.vector.memset(warm_sb[:], 0.0).then_inc(warm_sem, 1)
            for kind, i in dve:
                if kind == "cast":
                    s = i
                    eng.wait_ge(pe_sem, pe_count[("S1", s)])
                    nc.vector.tensor_copy(
                        vf[:, :, s % VFR, :],
                        vp[:, s % VPR, :],
                    ).then_inc(dve_sem, 1)
                    continue
                if kind == "fold":
                    p = i
                    # for the tail pairs the odd cast is in-stream on DVE
                    key = ("cast", 2 * p + 1)
                    if key in act_count:
                        eng.wait_ge(act_sem, act_count[key])
                    else:
                        eng.wait_ge(act_sem, act_count[("cast", 2 * p)])
                    f0 = (2 * p) % VFR
                    nc.vector.tensor_tensor(
                        vs_s[:, p % VSR, :, :],
                        vf[:, 0, f0 : f0 + 2, :],
                        vf[:, 1, f0 : f0 + 2, :],
                        add,
                    ).then_inc(dve_sem, 1)
                else:
                    q = i
                    eng.wait_ge(pe_sem, pe_count[("S2", q)])
                    if q >= P - 2:
                        src = vp[:, 2 * (q - (P - 2)) : 2 * (q - (P - 2)) + 2, :]
                    else:
                        src = op[:, 2 * (q % 2) : 2 * (q % 2) + 2, :]
                    nc.vector.tensor_copy(
                        os_[:, q, :, :], src
                    ).then_inc(dve_sem, 1)

    nc.compile()
    return nc


_NC_CACHE: bass.Bass | None = None


def _get_nc() -> bass.Bass:
    global _NC_CACHE
    if _NC_CACHE is None:
        _NC_CACHE = _build()
    return _NC_CACHE


def _make_in_maps(ip: np.ndarray) -> list[dict[str, np.ndarray]]:
    a = _dct_matrix()                                   # [256, 256] f32
    a_bf = a.astype(NP_BF16)
    unit_a = (
        a_bf.reshape(2, 128, 256).transpose(1, 0, 2).reshape(128, 512)
    )                                                   # [p, ki*256+j]
    unit_eo = np.zeros((128, 512), dtype=NP_BF16)
    unit_eo[:, 0:128] = a_bf[0:128, 0::2]               # E2[v, t']
    unit_eo[:, 128:256] = a_bf[0:128, 1::2]             # O2[v, t']
    unit_eo[:, 256:384] = -a_bf[0:128, 1::2]            # -O2
    in_maps = []
    for b in range(N_CORES):
        xb = ip[b].astype(NP_BF16)                      # [C, 256, 256]
        # w-permutation: cols 128.. hold w = 255..128
        xp = np.concatenate([xb[:, :, :128], xb[:, :, 128:][:, :, ::-1]], axis=2)
        # [s, ki, p, mi, c] -> [p, s, ki*256+mi*128+c]
        st = xp.reshape(C, 2, 128, 2, 128).transpose(2, 0, 1, 3, 4).reshape(128, C, 512)
        full = np.concatenate(
            [unit_a[:, None, :], unit_eo[:, None, :], st], axis=1
        )                                               # [128, 34, 512]
        in_maps.append({"x": np.ascontiguousarray(full)})
    return in_maps


def _unpack_out(results: list[dict[str, np.ndarray]]) -> np.ndarray:
    outs = []
    for b in range(N_CORES):
        o = np.asarray(results[b]["out"]).astype(np.float32)  # [128,16,2,512]
        o = o.reshape(128, P, 2, 2, 256)                # [t', pair, eo, sb, j]
        o = o.transpose(1, 3, 4, 0, 2).reshape(C, 256, 256)  # [s, j, w'=2t'+eo]
        outs.append(o)
    return np.stack(outs, axis=0)


def run(ip: np.ndarray, trace: bool = False):
    """Run the device kernel; returns (output, BassKernelResults)."""
    ip = np.asarray(ip)
    assert ip.shape == (N_CORES, C, 256, 256), ip.shape
    res = run_bass_kernel_spmd(
        _get_nc(), _make_in_maps(ip), core_ids=list(range(N_CORES)), trace=trace
    )
    return _unpack_out(res.results), res


def kernel(ip: np.ndarray) -> np.ndarray:
    out, _ = run(ip)
    return out


# revision 41
# speedup vs baseline: 1.1680x; 1.1680x over previous
"""2D DCT [8,32,256,256] on 8 TRN2 NeuronCores — raw Bass (no Tile).

Math: with A[m,k] = cos(pi*k*(m+0.5)/L)/L the 2D DCT per [256,256] slice is
    out = A^T @ X @ A
Stage 1: V = X^T A via 4 matmuls N=256 per slice (lhsT = X h-chunks,
rhs = A), one PSUM bank per slice. The host stages the second half of the
w columns REVERSED, so the bank holds
    vp[v, 0:256]   = v0 = V[v, j]        (v = 0..127)
    vp[v, 256:512] = v1 = V[255-v, j]
Stage 2 uses the DCT-II even/odd symmetry A[255-v, w'] = (-1)^w' A[v, w']:
    out[j, 2t']   = E2^T (v0 + v1),   E2[v,t'] = A[v, 2t']
    out[j, 2t'+1] = O2^T v0 - O2^T v1, O2[v,t'] = A[v, 2t'+1]
Per slice PAIR stage 2 is 3 matmuls of N=512 (contraction 128): the even
half consumes a DVE-folded s_w = v0+v1 (bf16, 2x-mode tensor_tensor); the
odd half does the subtract INSIDE PSUM accumulation using a staged -O2
(f32-exact, no fold needed). 1536+216 streamed PE columns per slice vs
2048 for the dense baseline, while the vector engines carry only
casts + one fold + out-evictions (~20us each, well under the PE's ~25us)
so the PE is self-paced — cross-engine hiccups don't propagate.

Pipeline per pair p (slices a=2p, b=2p+1):
    PE  S1(a), S1(b)          -> vp banks a%4, b%4  (4 MMs N=256 each)
    ACT cast(s) FD=512        vp bank -> vf[v0-group | v1-group] bf16
    DVE fold_s(p) FD=512 2x   vf v0,v1 -> vs_s (s_w pair, contiguous)
    PE  S2(p): E2^T s_w (N=512); O2^T v0pair - O2n^T v1pair (2 MMs N=512)
    DVE out-evict (ACT for pairs 3,9,15) op banks -> os bf16 FD=1024
    sync-ring DMA os -> DRAM (ACT DMAs the tail pair inline)

Wait discipline (waits break the LDWEIGHTS pull-ahead): PE block p =
[wait act>=cast(2p-3): vp two-agent guard, also implies S2(p-2)'s casts]
S1(2p) S1(2p+1) [wait dve>=out(p-4), which implies fold_s(p-2) by DVE
stream order] S2(p-2), LAG=2. ACT stream: cast(s) ascending. DVE:
fold_s(p) then out(p-2). The TAIL pairs (14, 15) write their stage-2
output into the by-then-free vp banks instead of the op ring, removing
the final out-evict -> S2 serialization from the kernel drain. Never two
agents on one PSUM bank concurrently (hard device crash) — every PSUM
reader/writer handoff above is sem-ordered.

Measured paces (this container, warm K=8/8 @2.4GHz): N=256 MM 109ns,
N=512 MM 216ns, ACT copy FD/1.2+143ns, DVE cast FD/0.96+65ns, DVE bf16
TT 2x FD/1.92+69ns. HAM: PE cold (1.2GHz) until ~3.4us of sustained
work — N_WARM garbage matmuls bridge the DMA head so real S1s start
warm. Dual-PSUM-operand tensor_tensor is ILLEGAL (walrus NCC_IBVF027);
DMA cannot read PSUM; matmul output must be f32 on TRN2 — these three
constraints shape the whole eviction pipeline.

Measured HW exec (neuron-profile, core 0): 43.4-44.7us across runs
(prior dense-baseline: 47.1-52.2us in the same container), rel err
3.7e-3. Breakdown at 43.4: ~1.4us counted boot, 3.5us HAM/DMA-head
warm, 25.9us compute window (24.3 PE-streaming floor + 1.4 stalls),
4.7us out-drain, 8.0us fixed runtime postamble (sem resets). Note the
device sometimes runs the PE at 2.0GHz (P0 power state) instead of
2.4GHz, which stretches the compute window ~20%.
"""

import numpy as np

import concourse.bacc as bacc
import concourse.bass as bass
import concourse.mybir as mybir
from concourse.bass_utils import run_bass_kernel_spmd

N_CORES = 8
C = 32                    # slices per core
P = 16                    # slice pairs per core
L = 256
BF16 = mybir.dt.bfloat16
F32 = mybir.dt.float32
NP_BF16 = mybir.dt.np(mybir.dt.bfloat16)

# staged input units: 0 = A, 1 = [E2|O2|O2n|pad], 2+s = slice s
IN_CHUNKS = [3, 1, 1, 1, 1, 1, 1, 1, 1, 2, 2, 3, 4, 5, 7]  # 34 units
OUT_CHUNKS = [3, 3, 3, 3, 2, 1]                   # pairs 0..14 on sync ring
TAIL_PAIR = 15                                    # pair 15 DMA'd from ACT
N_WARM = 32
VPR = 4                   # vp ring (banks) — slice s -> bank s%4
OPR = 4                   # op ring — pair p -> banks 2*(p%2), 2*(p%2)+1
VFR = 8                   # vf ring slots — slice s -> slot s%8
VSR = 6                   # vs_s ring — pair p -> slot p%6
LAG = 2                   # S2(p-LAG) in PE pair block p
OUT_ENG = ["act" if p == TAIL_PAIR else "dve" for p in range(P)]
# Tail odd casts on DVE were tried to parallelize the drain; DVE's
# strict-FIFO queue turned the hoisted cast's wait into head-of-line
# blocking and made the tail worse — keep all casts on ACT.
DVE_CASTS: set[int] = set()


def _dct_matrix() -> np.ndarray:
    m = np.arange(L, dtype=np.float64)
    k = np.arange(L, dtype=np.float64)
    a = np.cos(np.pi * np.outer(m + 0.5, k) / L) / L
    return a.astype(np.float32)


def _chunk_of_slice(s):
    u = s + 2
    c0 = 0
    for ci, n in enumerate(IN_CHUNKS):
        if u < c0 + n:
            return ci
        c0 += n
    raise AssertionError


def _schedules():
    """Per-engine op orders + completion counts (sem value when done)."""
    pe = []
    for p in range(P):
        pe.append(("S1", 2 * p))
        pe.append(("S1", 2 * p + 1))
        if p >= LAG:
            pe.append(("S2", p - LAG))
    for p in range(P - LAG, P):
        pe.append(("S2", p))
    pe_count = {o: i + 1 for i, o in enumerate(pe)}

    # ACT: casts ascending; out(q) placed right after cast(2q+3) so the
    # PE block's act>=cast(2p-3) wait transitively covers ACT outs.
    # The final pairs' odd casts (29, 31) run on DVE instead so the
    # kernel drain chain S1(31)->cast->fold->S2(15) isn't serialized
    # behind cast(30) on ACT.
    act = []
    for s in range(2 * P):
        if s in DVE_CASTS:
            continue
        act.append(("cast", s))
        if s >= 3 and s % 2 == 1:
            q = (s - 3) // 2
            if OUT_ENG[q] == "act":
                act.append(("out", q))
    for q in (P - 2, P - 1):
        if OUT_ENG[q] == "act":
            act.append(("out", q))
    act_count = {o: i + 1 for i, o in enumerate(act)}

    # DVE: fold_s(p) leads, out(p-2) trails, split per bank (odd bank
    # first) so S2's odd-half MMs gate only on the odd bank's recycle
    dve = []
    for p in range(P):
        if 2 * p + 1 in DVE_CASTS:
            dve.append(("cast", 2 * p + 1))
        dve.append(("fold", p))
        q = p - 2
        if q >= 0 and OUT_ENG[q] == "dve":
            dve.append(("out_o", q))
            dve.append(("out_e", q))
    for q in (P - 2, P - 1):
        if OUT_ENG[q] == "dve":
            dve.append(("out", q))
    dve_count = {o: i + 1 for i, o in enumerate(dve)}
    return pe, pe_count, act, act_count, dve, dve_count


def _build(sim: bool = False) -> bass.Bass:
    nc = bacc.Bacc()
    x = nc.declare_dram_parameter("x", [128, 2 + C, 512], BF16, isOutput=False)
    out = nc.declare_dram_parameter("out", [128, P, 2, 512], BF16, isOutput=True)

    pe, pe_count, act, act_count, dve, dve_count = _schedules()

    from contextlib import ExitStack

    ctx = ExitStack()
    with ctx:
        warm_sb = ctx.enter_context(nc.sbuf_tensor([128, 128], BF16))
        xs = ctx.enter_context(nc.sbuf_tensor([128, 2 + C, 512], BF16))
        # vf[:, 0, slot, :] = v0 of slice, vf[:, 1, slot, :] = v1
        vf = ctx.enter_context(nc.sbuf_tensor([128, 2, VFR, 256], BF16))
        vs_s = ctx.enter_context(nc.sbuf_tensor([128, VSR, 2, 256], BF16))
        os_ = ctx.enter_context(nc.sbuf_tensor([128, P, 2, 512], BF16))
        vp = ctx.enter_context(nc.psum_tensor([128, VPR, 512], F32))
        op = ctx.enter_context(nc.psum_tensor([128, OPR, 512], F32))

        in_sems = [
            ctx.enter_context(nc.semaphore(f"in_sem{i}"))
            for i in range(len(IN_CHUNKS))
        ]
        pe_sem = ctx.enter_context(nc.semaphore("pe_sem"))
        dve_sem = ctx.enter_context(nc.semaphore("dve_sem"))
        act_sem = ctx.enter_context(nc.semaphore("act_sem"))
        out_sem = ctx.enter_context(nc.semaphore("out_sem"))
        warm_sem = ctx.enter_context(nc.semaphore("warm_sem"))
        sem_of = {"dve": dve_sem, "act": act_sem}
        count_of = {"dve": dve_count, "act": act_count}

        block = ctx.enter_context(nc.Block())

        @block.sync
        def _(eng):
            u0 = 0
            for ci, n in enumerate(IN_CHUNKS):
                eng.dma_start(
                    xs[:, u0 : u0 + n, :], x[:, u0 : u0 + n, :]
                ).then_inc(in_sems[ci], 16)
                u0 += n
            def final_out_key(q):
                if OUT_ENG[q] == "dve" and q < P - 2:
                    return ("out_e", q)
                return ("out", q)

            c0 = 0
            for n in OUT_CHUNKS:
                for eng_name in ("dve", "act"):
                    need = max(
                        (
                            count_of[eng_name][final_out_key(q)]
                            for q in range(c0, c0 + n)
                            if OUT_ENG[q] == eng_name
                        ),
                        default=0,
                    )
                    if need:
                        eng.wait_ge(sem_of[eng_name], need)
                eng.dma_start(
                    out[:, c0 : c0 + n, :, :], os_[:, c0 : c0 + n, :, :]
                ).then_inc(out_sem, 16)
                c0 += n
            eng.wait_ge(out_sem, 16 * (len(OUT_CHUNKS) + 1))

        @block.tensor
        def _(eng):
            if sim:
                eng.wait_ge(warm_sem, 1)
            for _ in range(N_WARM):
                nc.tensor.matmul(
                    vp[:, 0, 0:128], warm_sb[:], warm_sb[:],
                    start=True, stop=True,
                )
            eng.wait_ge(in_sems[0], 16)
            seen_chunks = {0}
            for kind, i in pe:
                if kind == "S1":
                    s = i
                    ci = _chunk_of_slice(s)
                    if ci not in seen_chunks:
                        seen_chunks.add(ci)
                        eng.wait_ge(in_sems[ci], 16)
                    if s % 2 == 0 and s >= 4:
                        # vp two-agent guard: bank freed by cast(s-3);
                        # also implies everything S2(s//2 - 2) needs
                        # from the ACT stream
                        eng.wait_ge(act_sem, act_count[("cast", s - 3)])
                    r = s % VPR
                    for mi in range(2):
                        for ki in range(2):
                            mm = nc.tensor.matmul(
                                vp[:, r, mi * 256 : (mi + 1) * 256],
                                xs[:, 2 + s, ki * 256 + mi * 128 : ki * 256 + (mi + 1) * 128],
                                xs[:, 0, ki * 256 : (ki + 1) * 256],
                                start=(ki == 0),
                                stop=(ki == 1),
                            )
                    mm.then_inc(pe_sem, 1)
                else:
                    q = i
                    f0 = (2 * q) % VFR
                    if q >= P - 2:
                        # tail pairs write into the (now free) vp banks:
                        # no op-ring wait; fold(q) implies cast(2q+1)
                        # which implies the banks' casts are done — the
                        # wait must precede ALL writes here
                        eng.wait_ge(dve_sem, dve_count[("fold", q)])
                        ps, b0 = vp, 2 * (q - (P - 2))
                        late_fold_wait = False
                    else:
                        # odd-half MMs need only the casts (implied by
                        # this block's act guard) + the odd bank's
                        # recycle; the fold/even-bank waits come after
                        if q >= 2:
                            eng.wait_ge(dve_sem, dve_count[("out_o", q - 2)])
                        ps, b0 = op, 2 * (q % 2)
                        late_fold_wait = True
                    nc.tensor.matmul(
                        ps[:, b0 + 1, :],
                        xs[:, 1, 128:256],
                        vf[:, 0, f0 : f0 + 2, :],
                        start=True, stop=False,
                    )
                    nc.tensor.matmul(
                        ps[:, b0 + 1, :],
                        xs[:, 1, 256:384],
                        vf[:, 1, f0 : f0 + 2, :],
                        start=False, stop=True,
                    )
                    if late_fold_wait:
                        # out_e(q-2) sits after fold(q) in the DVE
                        # stream, so one wait covers both
                        eng.wait_ge(
                            dve_sem,
                            dve_count[("out_e", q - 2)]
                            if q >= 2
                            else dve_count[("fold", q)],
                        )
                    mm = nc.tensor.matmul(
                        ps[:, b0, :],
                        xs[:, 1, 0:128],
                        vs_s[:, q % VSR, :, :],
                        start=True, stop=True,
                    )
                    mm.then_inc(pe_sem, 1)

        @block.scalar
        def _(eng):
            for kind, i in act:
                if kind == "cast":
                    s = i
                    eng.wait_ge(pe_sem, pe_count[("S1", s)])
                    cp = nc.scalar.copy(
                        vf[:, :, s % VFR, :],
                        vp[:, s % VPR, :],
                    )
                else:
                    q = i
                    eng.wait_ge(pe_sem, pe_count[("S2", q)])
                    if q >= P - 2:
                        src = vp[:, 2 * (q - (P - 2)) : 2 * (q - (P - 2)) + 2, :]
                    else:
                        src = op[:, 2 * (q % 2) : 2 * (q % 2) + 2, :]
                    cp = nc.scalar.copy(os_[:, q, :, :], src)
                cp.then_inc(act_sem, 1)
            # tail out-DMA for pair 15, in parallel with the sync ring's
            # pair-14 chunk; the DGE must not read os_ before the
            # eviction writes land, so wait on ACT's own eviction sem
            eng.wait_ge(act_sem, act_count[("out", P - 1)])
            eng.dma_start(
                out[:, P - 1, :, :], os_[:, P - 1, :, :]
            ).then_inc(out_sem, 16)

        @block.vector
        def _(eng):
            add = mybir.AluOpType.add
            if sim:
                nc.vector.memset(warm_sb[:], 0.0).then_inc(warm_sem, 1)
            for kind, i in dve:
                if kind == "cast":
                    s = i
                    eng.wait_ge(pe_sem, pe_count[("S1", s)])
                    nc.vector.tensor_copy(
                        vf[:, :, s % VFR, :],
                        vp[:, s % VPR, :],
                    ).then_inc(dve_sem, 1)
                    continue
                if kind == "fold":
                    p = i
                    # for the tail pairs the odd cast is in-stream on DVE
                    key = ("cast", 2 * p + 1)
                    if key in act_count:
                        eng.wait_ge(act_sem, act_count[key])
                    else:
                        eng.wait_ge(act_sem, act_count[("cast", 2 * p)])
                    f0 = (2 * p) % VFR
                    nc.vector.tensor_tensor(
                        vs_s[:, p % VSR, :, :],
                        vf[:, 0, f0 : f0 + 2, :],
                        vf[:, 1, f0 : f0 + 2, :],
                        add,
                    ).then_inc(dve_sem, 1)
                elif kind == "out":
                    q = i
                    eng.wait_ge(pe_sem, pe_count[("S2", q)])
                    src = vp[:, 2 * (q - (P - 2)) : 2 * (q - (P - 2)) + 2, :]
                    nc.vector.tensor_copy(
                        os_[:, q, :, :], src
                    ).then_inc(dve_sem, 1)
                else:
                    q = i
                    b0 = 2 * (q % 2)
                    if kind == "out_o":
                        eng.wait_ge(pe_sem, pe_count[("S2", q)])
                        nc.vector.tensor_copy(
                            os_[:, q, 1, :], op[:, b0 + 1, :]
                        ).then_inc(dve_sem, 1)
                    else:
                        nc.vector.tensor_copy(
                            os_[:, q, 0, :], op[:, b0, :]
                        ).then_inc(dve_sem, 1)

    nc.compile()
    return nc


_NC_CACHE: bass.Bass | None = None


def _get_nc() -> bass.Bass:
    global _NC_CACHE
    if _NC_CACHE is None:
        _NC_CACHE = _build()
    return _NC_CACHE


def _make_in_maps(ip: np.ndarray) -> list[dict[str, np.ndarray]]:
    a = _dct_matrix()                                   # [256, 256] f32
    a_bf = a.astype(NP_BF16)
    unit_a = (
        a_bf.reshape(2, 128, 256).transpose(1, 0, 2).reshape(128, 512)
    )                                                   # [p, ki*256+j]
    unit_eo = np.zeros((128, 512), dtype=NP_BF16)
    unit_eo[:, 0:128] = a_bf[0:128, 0::2]               # E2[v, t']
    unit_eo[:, 128:256] = a_bf[0:128, 1::2]             # O2[v, t']
    unit_eo[:, 256:384] = -a_bf[0:128, 1::2]            # -O2
    in_maps = []
    for b in range(N_CORES):
        xb = ip[b].astype(NP_BF16)                      # [C, 256, 256]
        # w-permutation: cols 128.. hold w = 255..128
        xp = np.concatenate([xb[:, :, :128], xb[:, :, 128:][:, :, ::-1]], axis=2)
        # [s, ki, p, mi, c] -> [p, s, ki*256+mi*128+c]
        st = xp.reshape(C, 2, 128, 2, 128).transpose(2, 0, 1, 3, 4).reshape(128, C, 512)
        full = np.concatenate(
            [unit_a[:, None, :], unit_eo[:, None, :], st], axis=1
        )                                               # [128, 34, 512]
        in_maps.append({"x": np.ascontiguousarray(full)})
    return in_maps


def _unpack_out(results: list[dict[str, np.ndarray]]) -> np.ndarray:
    outs = []
    for b in range(N_CORES):
        o = np.asarray(results[b]["out"]).astype(np.float32)  # [128,16,2,512]
        o = o.reshape(128, P, 2, 2, 256)                # [t', pair, eo, sb, j]
        o = o.transpose(1, 3, 4, 0, 2).reshape(C, 256, 256)  # [s, j, w'=2t'+eo]
        outs.append(o)
    return np.stack(outs, axis=0)


def run(ip: np.ndarray, trace: bool = False):
    """Run the device kernel; returns (output, BassKernelResults)."""
    ip = np.asarray(ip)
    assert ip.shape == (N_CORES, C, 256, 256), ip.shape
    res = run_bass_kernel_spmd(
        _get_nc(), _make_in_maps(ip), core_ids=list(range(N_CORES)), trace=trace
    )
    return _unpack_out(res.results), res


def kernel(ip: np.ndarray) -> np.ndarray:
    out, _ = run(ip)
    return out
